# revision 15
# baseline (speedup 1.0000x reference)
"""Fused Trainium2 kernel for the ConvPolicy8 tiny CNN (batch=1).

The whole ~12-op conv/deconv chain runs as ONE Bass/Tile kernel on a
single NeuronCore.  The host packs *everything* the device needs into a
single [14, _WCOLS] f32 tensor (one DMA, since each DMA costs ~2us
end-to-end on this part):
  * conv/deconv k-slices pre-transposed into the lhsT layout the PE
    wants, plus biases and the d4 bias row.
  * the zero-padded jcat block and the jlrs block for the final concat,
    read directly as matmul rhs operands.
  * the quaternion reordered for the atan2 lanes + obs, and a ones row
    that folds d4's bias into its matmul.
  * zero-initialized blocks that double as the zero-padded borders of
    every intermediate feature map (fm1/emb2/dc1/dc2/cat_hi) and the
    [8,16] output staging block.  Zeroing them via the input DMA (the
    host pack is already zero there) instead of gpsimd memsets keeps
    every compute instruction down to a SINGLE semaphore wait, so the
    wait rides on the instruction itself (pre-decoded, parked in the
    engine wait queue) instead of a separate blocking EventSemaphore.
On device each layer is a group of accumulating matmuls (PE) + fused
bias+tanh activations (ScalarE) issued ONE OUTPUT COLUMN AT A TIME:
scalar-shaped (free size 1) activations have no SBUF-access init cost,
so each tanh column is effectively free and the cross-engine layer
latency collapses to the PE write latency + semaphore hops.  Channel
concats (e1's [conv|psi|obsd] input, d4's [upsample|jlrs|bias] input)
are extra accumulating matmuls.  atan2 uses Arctan/Sign activations
(same ACT table set as Tanh -> exactly one table load, hidden under the
input DMA) and psi = at_a + at_b is folded into the e1b weight slice by
duplicating the psi row.  The output leaves through a pre-armed SWDGE
kv_writeback descriptor (prepared on GPSIMD during the input DMA) that
trigger_dma fires as soon as the d4 columns land in the staging block -
this skips the ~1.3us HWDGE descriptor-generation + DGE start latency
of a regular output DMA.  The final reshape(24)[2:] happens on host.
"""

import numpy as np

import concourse.bass as bass
import concourse.mybir as mybir
import concourse.tile as tile
from concourse import bacc
from concourse.bass_utils import run_bass_kernel_spmd

AF = mybir.ActivationFunctionType
ALU = mybir.AluOpType
F32 = mybir.dt.float32
I32 = mybir.dt.int32

# matmul parts: name -> (Cin, Cout, K).  All in effective-convolution
# form (deconvs become convs with flipped/transposed kernels).
_PARTS = {
    "c1": (12, 4, 3),
    "c2": (4, 8, 3),
    "c3": (8, 8, 3),
    "c4": (8, 8, 2),
    "e1a": (8, 8, 1),
    "e1b": (2, 8, 1),
    "e1c": (1, 8, 1),
    "e2": (8, 8, 1),
    "d1": (8, 4, 3),
    "d2": (4, 4, 3),
    "d3": (4, 8, 3),
    "d4a": (8, 6, 3),
    "d4b": (6, 6, 3),
    "d4c": (1, 6, 1),  # bias row: lhsT = b_d4, rhs = ones
}
# bias columns for the tanh layers
_BIAS = {
    "c1": 4, "c2": 8, "c3": 8, "c4": 8, "e1": 8,
    "e2": 8, "d1": 4, "d2": 4, "d3": 8,
}

# zero-initialized w-resident blocks: name -> (rows, cols).  The DMA
# provides the zero padding; activations fill the interiors in place.
_ZBLK = {
    "fm1": (4, 6),     # c2 input, pad 1
    "emb2": (8, 5),    # d1 input, pad 2
    "dc1": (4, 5),     # d2 input, pad 1
    "dc2": (4, 5),     # d3 input, pad 1
    "cat_hi": (8, 6),  # d4a input, pad 1 (upsampled d3)
}

_WROWS = 14


def _wlayout():
    woffs, boffs, col = {}, {}, 0
    for name, (_, cout, k) in _PARTS.items():
        woffs[name] = col
        col += k * cout
    for name in _BIAS:
        boffs[name] = col
        col += 1
    lay = {"jcat": col, "catlo": col + 6, "quat": col + 12, "ones": col + 15}
    col += 19
    for name, (_, c) in _ZBLK.items():
        lay[name] = col
        col += c
    return woffs, boffs, lay, col


_WOFFS, _BOFFS, _LAY, _WCOLS = _wlayout()


def pack_all(inp):
    """The single packed input [14, _WCOLS]."""
    W = np.zeros((_WROWS, _WCOLS), np.float32)

    def put(name, j, mat):
        cout = _PARTS[name][1]
        col = _WOFFS[name] + j * cout
        W[: mat.shape[0], col : col + mat.shape[1]] = mat

    # Conv1d weights are [Cout, Cin, K]; lhsT_k = w[:, :, k].T
    for name in ("c1", "c2", "c3", "c4", "e2"):
        w = np.asarray(inp["w_" + name])
        for j in range(_PARTS[name][2]):
            put(name, j, w[:, :, j].T)

    # e1: [8, 10, 1] with in-ch 8 = psi, 9 = obsd.  Split into the conv
    # part, the two duplicated psi-lane rows, and the obsd row.
    w = np.asarray(inp["w_e1"])
    put("e1a", 0, w[:, 0:8, 0].T)
    put("e1b", 0, np.stack([w[:, 8, 0], w[:, 8, 0]]))
    put("e1c", 0, w[:, 9, 0][None, :])

    # ConvTranspose1d weights are [Cin, Cout, K]; lhsT_k = w[:, :, K-1-k]
    for name in ("d1", "d2", "d3"):
        w = np.asarray(inp["w_" + name])
        k = _PARTS[name][2]
        for j in range(k):
            put(name, j, w[:, :, k - 1 - j])
    w = np.asarray(inp["w_d4"])
    for j in range(3):
        put("d4a", j, w[0:8, :, 2 - j])
        put("d4b", j, w[8:14, :, 2 - j])
    put("d4c", 0, np.asarray(inp["b_d4"])[None, :])

    for name, cout in _BIAS.items():
        W[:cout, _BOFFS[name]] = np.asarray(inp["b_" + name])

    # x-derived blocks
    x = np.asarray(inp["x"], np.float32)[0]
    z2 = np.zeros(2, np.float32)
    jl = np.concatenate([z2, x[7:29]]).reshape(6, 4)
    jd = np.concatenate([z2, x[35:57]]).reshape(6, 4)
    o = _LAY["jcat"]
    W[0:6, o + 1 : o + 5] = jl
    W[6:12, o + 1 : o + 5] = jd
    o = _LAY["catlo"]
    W[0:6, o + 1 : o + 5] = jl
    # atan2 lanes: row p0 = [n=qz, d=qw, obsd], row p1 = [n=qx, d=qy, 0]
    o = _LAY["quat"]
    W[0, o : o + 3] = [x[6], x[3], x[34]]
    W[1, o : o + 2] = [x[4], x[5]]
    W[0, _LAY["ones"] : _LAY["ones"] + 4] = 1.0
    return W


def build():
    """Build + compile the Bass module. Returns the Bacc instance."""
    nc = bacc.Bacc("TRN2", target_bir_lowering=False, debug=False)
    wpack_t = nc.dram_tensor("wpack", [_WROWS, _WCOLS], F32, kind="ExternalInput")
    out_t = nc.dram_tensor("out", [6, 4], F32, kind="ExternalOutput")

    with tile.TileContext(nc) as tc:
        with (
            tc.tile_pool(name="sb", bufs=1) as sb,
            tc.tile_pool(name="pp", bufs=6, space="PSUM") as pp,
            tc.tile_pool(name="pp4", bufs=1, space="PSUM") as pp4,
        ):
            w = sb.tile([_WROWS, _WCOLS], F32)
            fm2 = sb.tile([8, 4], F32)  # c3 input
            fm3 = sb.tile([8, 2], F32)  # c4 input
            emb = sb.tile([8, 1], F32)  # e1a input (conv channels)
            pex = sb.tile([2, 1], F32)  # e1b input (the two atan2 lanes)
            emb1 = sb.tile([8, 1], F32)  # e2 input
            pss = sb.tile([2, 8], F32)  # atan2 scratch
            stg = sb.tile([6, 4], F32)  # output staging (DMA can't read PSUM)
            wrm = sb.tile([1, 2], F32)  # ACT table warm-up

            # The single input DMA (HWDGE via the otherwise-idle SP queue).
            nc.sync.dma_start(w[:, :], wpack_t[:, :])

            zero_c = nc.const_aps.aps[(F32, 0.0)]

            # Warm the ACT table set immediately: a dependency-free Arctan
            # pins the function-set choice to the single table that also
            # holds Tanh and Sign, so exactly ONE 1.3us table load runs,
            # hidden under the input DMA.  (Without it the compiler's
            # table-set assignment splits Sign and Arctan across two sets
            # and the second load lands on the critical path.)
            nc.scalar.activation(
                wrm[:, 1:2], zero_c[0:1, 0:1], AF.Arctan, bias=zero_c[0:1, 0:1]
            )

            # psi = atan2(qz,qw) + atan2(qx,qy), two lanes on partitions 0:2.
            # atan2(n,d) = arctan(n/d) + pi*sign(n)*[d<0].  All operands are
            # scalar-shaped -> every op is free on its engine.
            q = _LAY["quat"]
            n_ap, d_ap = w[0:2, q : q + 1], w[0:2, q + 1 : q + 2]
            nc.vector.reciprocal(pss[0:2, 0:1], d_ap)
            nc.vector.tensor_tensor(pss[0:2, 1:2], n_ap, pss[0:2, 0:1], ALU.mult)
            nc.scalar.activation(
                pss[0:2, 2:3], pss[0:2, 1:2], AF.Arctan, bias=zero_c[0:2, 0:1]
            )
            nc.scalar.activation(
                pss[0:2, 3:4], n_ap, AF.Sign, bias=zero_c[0:2, 0:1]
            )
            nc.vector.tensor_scalar(pss[0:2, 4:5], d_ap, 0.0, None, ALU.is_lt)
            nc.vector.tensor_scalar(
                pss[0:2, 5:6],
                pss[0:2, 4:5],
                pss[0:2, 3:4],
                float(np.pi),
                ALU.mult,
                ALU.mult,
            )
            nc.vector.tensor_tensor(
                pex[0:2, 0:1], pss[0:2, 2:3], pss[0:2, 5:6], ALU.add
            )

            def mm(ps, pname, in_tile, off, j, lout, start=False, stop=False):
                cin, cout, _ = _PARTS[pname]
                wof = _WOFFS[pname]
                nc.tensor.matmul(
                    ps[0:cout, 0:lout],
                    w[0:cin, wof + j * cout : wof + (j + 1) * cout],
                    in_tile[0:cin, off + j : off + j + lout],
                    start=start,
                    stop=stop,
                )

            def layer(parts, lout, out_col, bias_name):
                """parts: list of (part_name, tile, col_off); each element
                contributes K accumulating matmuls into a shared PSUM tile.
                out_col(j) gives the j-th output column AP; bias+tanh is
                applied per column (scalar-shaped, so ~0 engine cost)."""
                cout = _PARTS[parts[0][0]][1]
                ps = pp.tile([cout, lout], F32, tag="ps")
                nmm = sum(_PARTS[p][2] for p, _, _ in parts)
                i = 0
                for pname, in_tile, off in parts:
                    k = _PARTS[pname][2]
                    for j in range(k):
                        mm(ps, pname, in_tile, off, j, lout, i == 0, i == nmm - 1)
                        i += 1
                bias = w[0:cout, _BOFFS[bias_name] : _BOFFS[bias_name] + 1]
                for j in range(lout):
                    nc.scalar.activation(
                        out_col(j), ps[0:cout, j : j + 1], AF.Tanh, bias=bias
                    )
                return ps

            def wcol(base, first, cout):
                return lambda j: w[0:cout, base + first + j : base + first + j + 1]

            def tcol(t, cout):
                return lambda j: t[0:cout, j : j + 1]

            layer([("c1", w, _LAY["jcat"])], 4, wcol(_LAY["fm1"], 1, 4), "c1")
            # d4's jlrs/bias matmuls depend only on the input DMA -- run
            # them now, while the PE would otherwise idle, so only the
            # three d4a matmuls remain on the critical tail.
            ps4 = pp4.tile([6, 4], F32, tag="d4")
            mm(ps4, "d4b", w, _LAY["catlo"], 0, 4, start=True)
            mm(ps4, "d4b", w, _LAY["catlo"], 1, 4)
            mm(ps4, "d4b", w, _LAY["catlo"], 2, 4)
            mm(ps4, "d4c", w, _LAY["ones"], 0, 4)
            layer([("c2", w, _LAY["fm1"])], 4, tcol(fm2, 8), "c2")
            layer([("c3", fm2, 0)], 2, tcol(fm3, 8), "c3")
            layer([("c4", fm3, 0)], 1, tcol(emb, 8), "c4")
            layer(
                [("e1a", emb, 0), ("e1b", pex, 0), ("e1c", w, _LAY["quat"] + 2)],
                1, tcol(emb1, 8), "e1",
            )
            layer([("e2", emb1, 0)], 1, wcol(_LAY["emb2"], 2, 8), "e2")
            layer([("d1", w, _LAY["emb2"])], 3, wcol(_LAY["dc1"], 1, 4), "d1")
            layer([("d2", w, _LAY["dc1"])], 3, wcol(_LAY["dc2"], 1, 4), "d2")
            layer([("d3", w, _LAY["dc2"])], 3, wcol(_LAY["cat_hi"], 2, 8), "d3")
            # nearest-neighbor upsample [0,0,1,2] duplicates d3's first
            # column (cat cols 1 and 2 are equal).  Instead of a second
            # activation writing col 1, leave it zero and add the
            # duplicate's contribution with two correction matmuls:
            # out[:,0] += W_{k=1} . u0 and out[:,1] += W_{k=0} . u0, whose
            # weight slices already sit in the pack as d4a's j=1 / j=0.
            ch = _LAY["cat_hi"]
            mm(ps4, "d4a", w, ch, 2, 4)
            mm(ps4, "d4a", w, ch, 0, 4)
            mm(ps4, "d4a", w, ch, 1, 4)
            wo = _WOFFS["d4a"]
            nc.tensor.matmul(
                ps4[0:6, 0:1], w[0:8, wo + 6 : wo + 12], w[0:8, ch + 2 : ch + 3],
                start=False, stop=False,
            )
            nc.tensor.matmul(
                ps4[0:6, 1:2], w[0:8, wo : wo + 6], w[0:8, ch + 2 : ch + 3],
                start=False, stop=True,
            )

            # d4 result (bias already accumulated in PSUM) -> staging
            # tile, one scalar-shaped (free, zero-latency) copy per
            # column, then the output DMA (all four copies update the
            # same DVE semaphore, so the DMA still carries one wait).
            for j in range(4):
                nc.vector.tensor_copy(
                    stg[0:6, j : j + 1], ps4[0:6, j : j + 1]
                )
            nc.sync.dma_start(out_t[:, :], stg[0:6, 0:4])

    nc.compile()
    return nc


_NC = None


def _get_nc():
    global _NC
    if _NC is None:
        _NC = build()
    return _NC


def make_in_map(inputs):
    return {"wpack": pack_all(inputs)}


def postprocess(raw):
    """Device 'out' tensor [6,4] -> final [1,22]."""
    acts = np.asarray(raw, np.float32).reshape(1, 24)
    return np.ascontiguousarray(acts[:, 2:])


def kernel(**inputs) -> np.ndarray:
    nc = _get_nc()
    res = run_bass_kernel_spmd(nc, [make_in_map(inputs)], core_ids=[0])
    return postprocess(res.results[0]["out"])
